# revision 39
# baseline (speedup 1.0000x reference)
"""Trainium2 Bass kernel for nn_GCNGraphClassifier (NNConv edge-MLP GNN).

Host-side algebra (exact):
  - node features are a single broadcast row (node_emb), so x[src] is the same
    vector for every edge -> fold the per-edge einsum into w3:
        w3_eff[k,h] = sum_i emb[i] * w3[k, i*HID+h]
    and the scatter-add commutes with that linear map, so we scatter h2
    (the 64-dim second MLP layer output) and apply w3_eff per node.
  - out[n] = relu(agg_h2[n] @ w3_eff + deg[n]*b3_eff + (emb@root_w + conv_b))
  - pooling: one-hot(graph) matmul accumulating sums; division by counts is
    applied exactly via a host-computed 1/max(count,1) matrix.

Sharding: nodes are split into 8 contiguous ranges of 2560 (20 tiles of 128).
Edges (sorted by dst) follow their destination node. Each core scatters its
edges into its node range; graph pooling partial sums are AllReduced and the
tiny classifier head is computed (replicated) on every core.
"""

import numpy as np
import ml_dtypes
from contextlib import ExitStack

import concourse.bacc as bacc
import concourse.bass as bass
import concourse.mybir as mybir
import concourse.tile as tile
from concourse.bass_utils import run_bass_kernel_spmd

F32 = mybir.dt.float32
F16 = mybir.dt.float16
AF = mybir.ActivationFunctionType
ALU = mybir.AluOpType

# problem constants (hardcoded per contract)
N_NODES = 20000
N_EDGES = 200000
N_GRAPHS = 128
IN_DIM = 16
EDGE_DIM = 8
HID = 64
OUT_DIM = 10

NCORE = 8
NS = 2560          # nodes per core
NT = NS // 128     # 20 node tiles per core
P = 128

_BUILD_CACHE = {}


def _preprocess(inputs):
    ea = np.asarray(inputs["edge_attr"], dtype=np.float32)          # [E, 8]
    ei = np.asarray(inputs["edge_index"]).astype(np.int64)          # [2, E]
    batch = np.asarray(inputs["batch"]).astype(np.int64)            # [N]
    node_emb = np.asarray(inputs["node_emb"], dtype=np.float32)
    w1 = np.asarray(inputs["w1"], dtype=np.float32)
    b1 = np.asarray(inputs["b1"], dtype=np.float32)
    w2 = np.asarray(inputs["w2"], dtype=np.float32)
    b2 = np.asarray(inputs["b2"], dtype=np.float32)
    w3 = np.asarray(inputs["w3"], dtype=np.float32)
    b3 = np.asarray(inputs["b3"], dtype=np.float32)
    root_w = np.asarray(inputs["root_w"], dtype=np.float32)
    conv_b = np.asarray(inputs["conv_b"], dtype=np.float32)
    l1_w = np.asarray(inputs["l1_w"], dtype=np.float32)
    l1_b = np.asarray(inputs["l1_b"], dtype=np.float32)
    l2_w = np.asarray(inputs["l2_w"], dtype=np.float32)
    l2_b = np.asarray(inputs["l2_b"], dtype=np.float32)

    emb = node_emb[0]                                               # [16]
    w3_eff = np.einsum("i,kih->kh", emb, w3.reshape(HID, IN_DIM, HID))
    b3_eff = emb @ b3.reshape(IN_DIM, HID)                          # [64]
    r0 = emb @ root_w + conv_b                                      # [64]

    dst = ei[1]
    order = np.argsort(dst, kind="stable")
    dst_s = dst[order]
    ea_s = ea[order]

    # edge-balanced groups: cut the sorted node list into <=160 contiguous
    # groups of <=128 nodes, each capped at LIMIT edges, so every tile
    # round needs the same (minimal) chunk count; LPT-match across cores.
    n_all = NCORE * NS
    ndeg = np.bincount(dst, minlength=n_all).astype(np.int64)
    grp_bounds = None
    for limit in (1280, 1344, 1408, 1536):
        bounds = [0]
        acc = 0
        for n in range(N_NODES):        # fake nodes need no group
            c = int(ndeg[n])
            if (n - bounds[-1]) >= P or (acc + c > limit and n > bounds[-1]):
                bounds.append(n)
                acc = 0
            acc += c
        if len(bounds) <= NCORE * NT:
            grp_bounds = bounds
            break
    if grp_bounds is None:  # fallback: fixed 128-node groups
        grp_bounds = list(range(0, n_all, P))
    while len(grp_bounds) < NCORE * NT:
        grp_bounds.append(N_NODES)
    grp_bounds.append(N_NODES)
    grp_bounds = np.asarray(grp_bounds[: NCORE * NT + 1], dtype=np.int64)
    node_edge_start = np.concatenate([[0], np.cumsum(ndeg)])
    counts = (node_edge_start[grp_bounds[1:]]
              - node_edge_start[grp_bounds[:-1]])                   # [160]
    tile_start = node_edge_start[grp_bounds[:-1]]
    tile_start = np.concatenate([tile_start, [node_edge_start[n_all]]])
    order_g = np.argsort(-counts, kind="stable")                    # desc by edges
    # round r gets groups ranked [8r, 8r+8); core c takes the c-th of them
    assign = order_g.reshape(NT, NCORE)                             # [t, c] -> group
    counts_tc = counts[assign]                                      # [NT, NCORE]
    CH = np.maximum(1, np.ceil(counts_tc / P)).max(axis=1).astype(np.int64)  # [NT]
    c_total = int(CH.sum())
    pad8 = (-c_total) % 8
    CH[int(np.argmax(CH))] += 0  # keep shape
    CH[NT - 1] += pad8
    c_total += pad8
    e_pad = P * c_total
    chunk_base = np.concatenate([[0], np.cumsum(CH)])               # [NT+1]

    # per-core padded edge arrays; attr16 stored pre-transposed [16, e_pad]
    attr16 = np.zeros((NCORE, 16, e_pad), dtype=np.float16)
    lidx = np.full((NCORE, P, c_total), 300.0, dtype=np.float32)
    for c in range(NCORE):
        for t in range(NT):
            g = int(assign[t, c])
            s = int(node_edge_start[grp_bounds[g]])
            e = s + int(counts[g])
            m = e - s
            if m == 0:
                continue
            slot0 = P * int(chunk_base[t])
            attr16[c, 0:8, slot0 : slot0 + m] = ea_s[s:e].T.astype(np.float16)
            li = (dst_s[s:e] - int(grp_bounds[g])).astype(np.float32)
            idx = np.arange(m)
            lidx[c, idx % P, int(chunk_base[t]) + idx // P] = li

    deg_pad = np.zeros(n_all, np.float32)
    deg_pad[:N_NODES] = np.bincount(dst, minlength=N_NODES).astype(np.float32)[:N_NODES]
    batch_pad = np.full(n_all, 999.0, np.float32)
    batch_pad[:N_NODES] = batch.astype(np.float32)
    degrows = np.zeros((NCORE, NT, P), np.float32)
    batchcols = np.full((NCORE, P, NT), 999.0, np.float32)
    for c in range(NCORE):
        for t in range(NT):
            g = int(assign[t, c])
            n0, n1 = int(grp_bounds[g]), int(grp_bounds[g + 1])
            w = n1 - n0
            degrows[c, t, 0:w] = deg_pad[n0:n1]
            batchcols[c, 0:w, t] = batch_pad[n0:n1]

    gcounts = np.bincount(batch, minlength=N_GRAPHS).astype(np.float32)
    inv_counts = 1.0 / np.maximum(gcounts, 1.0)
    icmat = np.ascontiguousarray(np.broadcast_to(inv_counts[None, :], (HID, P))).astype(np.float32)

    iota128 = np.ascontiguousarray(
        np.broadcast_to(np.arange(P, dtype=np.float16)[None, :], (P, P))
    )
    iota_rep = np.ascontiguousarray(
        np.broadcast_to(
            np.tile(np.arange(P, dtype=np.float16), 8)[None, :], (P, 8 * P))
    )

    has_b2 = bool(np.any(b2 != 0.0))
    has_b1 = bool(np.any(b1 != 0.0))
    # block-diagonal double-w1: one K=32 matmul computes h1 for two
    # 512-edge blocks into the two partition halves of one PSUM tile
    w1s = np.zeros((32, P), dtype=np.float16)
    w1s[0:8, 0:HID] = w1.astype(np.float16)
    w1s[16:24, HID:P] = w1.astype(np.float16)
    # b1 bias applied twice (both partition halves of the packed L1 psum)
    b1two = np.concatenate([b1, b1]).astype(np.float32)[:, None]    # [128, 1]
    # block-diagonal w2: one K=128 matmul computes h2 for two chunks
    w2blk = np.zeros((P, P), np.float32)
    w2blk[0:HID, 0:HID] = w2
    w2blk[HID:P, HID:P] = w2
    b2bcast = np.ascontiguousarray(
        np.broadcast_to(np.tile(b2, 8)[None, :], (P, 8 * HID))).astype(np.float32)
    # rows: 0-63 w3_eff, 64 <- r0 (ones row), 65 <- b3_eff (deg row)
    w3aug = np.concatenate([w3_eff, r0[None, :], b3_eff[None, :]], axis=0)  # [66, 64]
    l1aug = np.concatenate([l1_w, l1_b[None, :]], axis=0)           # [65, 128]
    l2bcol = np.ascontiguousarray(l2_b[:, None])                    # [10, 1]

    shared = {
        "iota128": iota128,
        "iota_rep": iota_rep,
        "w1s": w1s,
        "b1two": b1two,
        "w2blk": w2blk.astype(np.float16),
        "b2bcast": b2bcast,
        "w3aug": w3aug.astype(np.float16),
        "l1aug": l1aug.astype(np.float16),
        "l2w": l2_w.astype(np.float16),
        "l2bcol": l2bcol.astype(np.float32),
        "icmat": icmat,
    }
    in_maps = []
    for c in range(NCORE):
        m = dict(shared)
        # pair-stack: rows 0-15 = even 512-block, rows 16-31 = odd block
        a = attr16[c].reshape(16, e_pad // 1024, 2, 512)
        m["attr16"] = np.ascontiguousarray(
            a.transpose(2, 0, 1, 3).reshape(32, e_pad // 2))
        m["lidx"] = np.ascontiguousarray(lidx[c]).astype(np.float16)
        m["degrows"] = np.ascontiguousarray(degrows[c]).astype(np.float16)
        m["batchcols"] = np.ascontiguousarray(batchcols[c]).astype(np.float16)
        in_maps.append(m)
    return in_maps, (tuple(int(x) for x in CH), has_b2, has_b1)


def _build(ch_key):
    if ch_key in _BUILD_CACHE:
        return _BUILD_CACHE[ch_key]
    CH = list(ch_key[0])
    has_b2 = ch_key[1]
    has_b1 = ch_key[2]
    c_total = sum(CH)
    e_pad = P * c_total
    nb1 = e_pad // 512

    nc = bacc.Bacc("TRN2", target_bir_lowering=False, debug=False,
                   num_devices=NCORE)

    d_attr = nc.dram_tensor("attr16", [32, e_pad // 2], F16, kind="ExternalInput")
    d_lidx = nc.dram_tensor("lidx", [P, c_total], F16, kind="ExternalInput")
    d_deg = nc.dram_tensor("degrows", [NT, P], F16, kind="ExternalInput")
    d_bat = nc.dram_tensor("batchcols", [P, NT], F16, kind="ExternalInput")
    d_iota = nc.dram_tensor("iota128", [P, P], F16, kind="ExternalInput")
    d_iotar = nc.dram_tensor("iota_rep", [P, 8 * P], F16, kind="ExternalInput")
    d_w1 = nc.dram_tensor("w1s", [32, P], F16, kind="ExternalInput")
    d_b1 = nc.dram_tensor("b1two", [P, 1], F32, kind="ExternalInput")
    d_w2 = nc.dram_tensor("w2blk", [P, P], F16, kind="ExternalInput")
    d_b2b = nc.dram_tensor("b2bcast", [P, 8 * HID], F32, kind="ExternalInput")
    d_w3 = nc.dram_tensor("w3aug", [HID + 2, HID], F16, kind="ExternalInput")
    d_l1 = nc.dram_tensor("l1aug", [HID + 1, 2 * HID], F16, kind="ExternalInput")
    d_l2 = nc.dram_tensor("l2w", [2 * HID, OUT_DIM], F16, kind="ExternalInput")
    d_l2b = nc.dram_tensor("l2bcol", [OUT_DIM, 1], F32, kind="ExternalInput")
    d_ic = nc.dram_tensor("icmat", [HID, P], F32, kind="ExternalInput")
    d_ccina = nc.dram_tensor("ccina", [HID, P], F16)
    d_ccouta = nc.dram_tensor("ccouta", [HID, P], F16, addr_space="Shared")
    d_ccinb = nc.dram_tensor("ccinb", [HID, P], F32)
    d_ccoutb = nc.dram_tensor("ccoutb", [HID, P], F32, addr_space="Shared")
    d_out = nc.dram_tensor("logitsT", [OUT_DIM, P], F32, kind="ExternalOutput")

    with tile.TileContext(nc) as tc, ExitStack() as ctx:
        const = ctx.enter_context(tc.tile_pool(name="const", bufs=1))

        attrT = const.tile([32, e_pad // 2], F16)
        # packed: rows 0-63 = h1 of even 512-block, rows 64-127 = odd block
        h1T2 = const.tile([P, e_pad // 2], F16)
        lidx_sb = const.tile([P, c_total], F16)
        iota_sb = const.tile([P, P], F16)
        iotar_sb = const.tile([P, 8 * P], F16)
        bat_sb = const.tile([P, NT], F16)
        w1_sb = const.tile([32, P], F16)
        b1_sb = const.tile([P, 1], F32)
        w2_sb = const.tile([P, P], F16)
        b2b_sb = const.tile([P, 8 * HID], F32)
        w3_sb = const.tile([HID + 2, HID], F16)
        l1_sb = const.tile([HID + 1, 2 * HID], F16)
        l2_sb = const.tile([2 * HID, OUT_DIM], F16)
        l2b_sb = const.tile([OUT_DIM, 1], F32)
        ic_sb = const.tile([HID, P], F32)
        # two persistent agga tiles (double-buffered by hand) so the ones
        # row is written once, not per node tile
        agga2 = [const.tile([HID + 2, P], F16, name=f"agga{i}", tag=f"agga{i}") for i in range(2)]

        n_sp = 8
        sp = e_pad // 2 // n_sp
        for s in range(n_sp):
            nc.sync.dma_start(attrT[:, s * sp : (s + 1) * sp],
                              d_attr[:, s * sp : (s + 1) * sp])
        nc.sync.dma_start(lidx_sb[:], d_lidx[:])
        nc.sync.dma_start(iota_sb[:], d_iota[:])
        nc.sync.dma_start(iotar_sb[:], d_iotar[:])
        nc.sync.dma_start(bat_sb[:], d_bat[:])
        nc.sync.dma_start(w1_sb[:], d_w1[:])
        nc.sync.dma_start(b1_sb[:], d_b1[:])
        nc.sync.dma_start(w2_sb[:], d_w2[:])
        if has_b2:
            nc.sync.dma_start(b2b_sb[:], d_b2b[:])
        nc.sync.dma_start(w3_sb[:], d_w3[:])
        nc.sync.dma_start(l1_sb[:], d_l1[:])
        nc.sync.dma_start(l2_sb[:], d_l2[:])
        nc.sync.dma_start(l2b_sb[:], d_l2b[:])
        nc.sync.dma_start(ic_sb[:], d_ic[:])
        nc.vector.memset(agga2[0][HID : HID + 1, :], 1.0)
        nc.vector.memset(agga2[1][HID : HID + 1, :], 1.0)
        # warmup operand needs no DMA
        warm_sb = const.tile([P, P], F16)
        nc.vector.memset(warm_sb[:], 0.125)


        with (
            tc.tile_pool(name="ps_pool", bufs=1, space="PSUM") as ps_pool,
            tc.tile_pool(name="ps_h1", bufs=2, space="PSUM") as ps_h1,
            tc.tile_pool(name="ps_h2", bufs=2, space="PSUM") as ps_h2,
            tc.tile_pool(name="ps_agg", bufs=2, space="PSUM") as ps_agg,
            tc.tile_pool(name="ps_out3", bufs=1, space="PSUM") as ps_out3,
            tc.tile_pool(name="work", bufs=3) as work,
            tc.tile_pool(name="ohpool", bufs=4) as ohpool,
        ):
            # all 20 pooling one-hots in one wide compare
            ohg_all = const.tile([P, NT * P], F16)
            nc.vector.tensor_tensor(
                ohg_all[:].rearrange("p (a j) -> p a j", a=NT),
                iota_sb[:].rearrange("p (a j) -> p a j", a=1).broadcast_to((P, NT, P)),
                bat_sb[:, 0:NT].broadcast_to((P, NT, P)),
                op=ALU.is_equal)

            # ---- PE warm-up: keep HAM busy while input DMAs land ----
            for wi in range(16):
                wps = ps_h1.tile([P, 512], F32, name="wps", tag="h1ps")
                nc.tensor.matmul(wps[:, 0:P], warm_sb[:], warm_sb[:],
                                 start=True, stop=True)

            # ---- L1 packed: block-diag w1 computes two 512-blocks/matmul ----
            for b in range(nb1 // 2):
                h1ps = ps_h1.tile([P, 512], F32, name="h1ps", tag="h1ps")
                nc.tensor.matmul(h1ps[:], w1_sb[:],
                                 attrT[:, b * 512 : (b + 1) * 512],
                                 start=True, stop=True)
                if has_b1 or b % 2 == 0:
                    nc.scalar.activation(h1T2[:, b * 512 : (b + 1) * 512],
                                         h1ps[:], AF.Relu, bias=b1_sb[:, 0:1])
                else:
                    nc.vector.tensor_scalar_max(
                        h1T2[:, b * 512 : (b + 1) * 512], h1ps[:], 0.0)

            # ---- L2 + scatter, per node tile ----
            n_groups = c_total // 8
            h2grp = [None] * n_groups
            # L2 in groups of 8 chunks sharing one PSUM bank & one relu op
            # one matmul per chunk-PAIR (chunks 8p+j and 8p+4+j share a
            # column window of h1T2; block-diag w2 separates them)
            for grp in range(n_groups):
                h2ps = ps_h2.tile([P, 512], F32)
                p_base = grp  # group == h1T2 512-col window == chunk pair block
                for j in range(4):
                    col = grp * 512 + j * P
                    nc.tensor.matmul(
                        h2ps[:, j * P : (j + 1) * P],
                        h1T2[:, col : col + P],
                        w2_sb[:],
                        start=True, stop=True)
                h2sb = work.tile([P, 512], F16, tag="h2sb")
                if has_b2:
                    h2tmp = work.tile([P, 512], F32, tag="h2tmp")
                    nc.vector.tensor_tensor(h2tmp[:], h2ps[:], b2b_sb[:],
                                            op=ALU.add)
                    nc.vector.tensor_scalar_max(h2sb[:], h2tmp[:], 0.0)
                else:
                    nc.scalar.activation(h2sb[:], h2ps[:], AF.Relu)
                h2grp[grp] = h2sb

            poolps = ps_pool.tile([HID, P], F32)
            n_ohb = c_total // 8
            ohbatch = [None] * n_ohb
            for ob in range(n_ohb):
                ohb = ohpool.tile([P, 8 * P], F16, tag="oh")
                nc.vector.tensor_tensor(
                    ohb[:].rearrange("p (a j) -> p a j", a=8),
                    iotar_sb[:].rearrange("p (a j) -> p a j", a=8),
                    lidx_sb[:, 8 * ob : 8 * ob + 8].broadcast_to((P, 8, P)),
                    op=ALU.is_equal)
                ohbatch[ob] = ohb

            kglob = 0
            for t in range(NT):
                aggps = ps_agg.tile([HID, P], F32)
                for j in range(CH[t]):
                    k = kglob + j
                    oh = ohbatch[k // 8][:, (k % 8) * P : (k % 8 + 1) * P]
                    h2sb = h2grp[k // 8]
                    hcol = P * (k % 4) + HID * ((k // 4) % 2)
                    nc.tensor.matmul(
                        aggps[:], h2sb[:, hcol : hcol + HID], oh[:],
                        start=(j == 0), stop=(j == CH[t] - 1))
                kglob += CH[t]

                # epilogue for node tile t
                agga = agga2[t % 2]
                nc.scalar.copy(agga[0:HID, :], aggps[:])
                nc.sync.dma_start(agga[HID + 1 : HID + 2, :],
                                  d_deg[t : t + 1, :])
                out3ps = ps_out3.tile([P, HID], F32, name="out3ps", tag="o3")
                nc.tensor.matmul(out3ps[:], agga[:], w3_sb[:], start=True, stop=True)
                outsb = work.tile([P, HID], F16, tag="outsb")
                nc.scalar.activation(outsb[:], out3ps[:], AF.Relu)
                nc.tensor.matmul(poolps[:], outsb[:],
                                 ohg_all[:, t * P : (t + 1) * P],
                                 start=(t == 0), stop=(t == NT - 1))

        pooled_sb = const.tile([HID, P], F16)
        nc.vector.tensor_copy(pooled_sb[:], poolps[:])
        nc.sync.dma_start(d_ccina[:], pooled_sb[:])
        nc.gpsimd.collective_compute(
            "AllReduce", ALU.add,
            replica_groups=[list(range(NCORE))],
            ins=[d_ccina[:]], outs=[d_ccouta[:]])

        with tc.tile_pool(name="ps_head", bufs=2, space="PSUM") as ps_head:
            pc_raw = const.tile([HID, P], F16)
            nc.sync.dma_start(pc_raw[:], d_ccouta[:])
            pc_aug = const.tile([HID + 1, P], F16)
            nc.vector.tensor_tensor(pc_aug[0:HID, :], pc_raw[:], ic_sb[:],
                                    op=ALU.mult)
            nc.vector.memset(pc_aug[HID : HID + 1, :], 1.0)
            zps = ps_head.tile([2 * HID, P], F32)
            nc.tensor.matmul(zps[:], l1_sb[:], pc_aug[:], start=True, stop=True)
            zsb = const.tile([2 * HID, P], F16)
            nc.scalar.activation(zsb[:], zps[:], AF.Relu)
            lps = ps_head.tile([OUT_DIM, P], F32)
            nc.tensor.matmul(lps[:], l2_sb[:], zsb[:], start=True, stop=True)
            lsb = const.tile([OUT_DIM, P], F32)
            nc.vector.tensor_scalar(lsb[:], lps[:], l2b_sb[:, 0:1], None,
                                    op0=ALU.add)
            nc.sync.dma_start(d_out[:], lsb[:])

    nc.compile()
    _BUILD_CACHE[ch_key] = nc
    return nc


def _run(inputs, trace=False, trace_cores=None, **kwargs):
    in_maps, ch_key = _preprocess(inputs)
    nc = _build(ch_key)
    res = run_bass_kernel_spmd(
        nc, in_maps, core_ids=list(range(NCORE)),
        trace=trace, trace_cores=trace_cores, **kwargs)
    return res


def kernel(**inputs):
    res = _run(inputs)
    logitsT = res.results[0]["logitsT"]
    return np.ascontiguousarray(logitsT.T).astype(np.float32)


# revision 40
# speedup vs baseline: 1.1658x; 1.1658x over previous
"""Trainium2 Bass kernel for nn_GCNGraphClassifier (NNConv edge-MLP GNN).

Host-side algebra (exact):
  - node features are a single broadcast row (node_emb), so x[src] is the same
    vector for every edge -> fold the per-edge einsum into w3:
        w3_eff[k,h] = sum_i emb[i] * w3[k, i*HID+h]
    and the scatter-add commutes with that linear map, so we scatter h2
    (the 64-dim second MLP layer output) and apply w3_eff per node.
  - out[n] = relu(agg_h2[n] @ w3_eff + deg[n]*b3_eff + (emb@root_w + conv_b))
  - pooling: one-hot(graph) matmul accumulating sums; division by counts is
    applied exactly via a host-computed 1/max(count,1) matrix.

Sharding: nodes are split into 8 contiguous ranges of 2560 (20 tiles of 128).
Edges (sorted by dst) follow their destination node. Each core scatters its
edges into its node range; graph pooling partial sums are AllReduced and the
tiny classifier head is computed (replicated) on every core.
"""

import numpy as np
import ml_dtypes
from contextlib import ExitStack

import concourse.bacc as bacc
import concourse.bass as bass
import concourse.mybir as mybir
import concourse.tile as tile
from concourse.bass_utils import run_bass_kernel_spmd

F32 = mybir.dt.float32
F16 = mybir.dt.float16
AF = mybir.ActivationFunctionType
ALU = mybir.AluOpType

# problem constants (hardcoded per contract)
N_NODES = 20000
N_EDGES = 200000
N_GRAPHS = 128
IN_DIM = 16
EDGE_DIM = 8
HID = 64
OUT_DIM = 10

NCORE = 8
NS = 2560          # nodes per core
NT = NS // 128     # 20 node tiles per core
P = 128

_BUILD_CACHE = {}


def _preprocess(inputs):
    ea = np.asarray(inputs["edge_attr"], dtype=np.float32)          # [E, 8]
    ei = np.asarray(inputs["edge_index"]).astype(np.int64)          # [2, E]
    batch = np.asarray(inputs["batch"]).astype(np.int64)            # [N]
    node_emb = np.asarray(inputs["node_emb"], dtype=np.float32)
    w1 = np.asarray(inputs["w1"], dtype=np.float32)
    b1 = np.asarray(inputs["b1"], dtype=np.float32)
    w2 = np.asarray(inputs["w2"], dtype=np.float32)
    b2 = np.asarray(inputs["b2"], dtype=np.float32)
    w3 = np.asarray(inputs["w3"], dtype=np.float32)
    b3 = np.asarray(inputs["b3"], dtype=np.float32)
    root_w = np.asarray(inputs["root_w"], dtype=np.float32)
    conv_b = np.asarray(inputs["conv_b"], dtype=np.float32)
    l1_w = np.asarray(inputs["l1_w"], dtype=np.float32)
    l1_b = np.asarray(inputs["l1_b"], dtype=np.float32)
    l2_w = np.asarray(inputs["l2_w"], dtype=np.float32)
    l2_b = np.asarray(inputs["l2_b"], dtype=np.float32)

    emb = node_emb[0]                                               # [16]
    w3_eff = np.einsum("i,kih->kh", emb, w3.reshape(HID, IN_DIM, HID))
    b3_eff = emb @ b3.reshape(IN_DIM, HID)                          # [64]
    r0 = emb @ root_w + conv_b                                      # [64]

    dst = ei[1]
    order = np.argsort(dst, kind="stable")
    dst_s = dst[order]
    ea_s = ea[order]

    # edge-balanced groups: cut the sorted node list into <=160 contiguous
    # groups of <=128 nodes, each capped at LIMIT edges, so every tile
    # round needs the same (minimal) chunk count; LPT-match across cores.
    n_all = NCORE * NS
    ndeg = np.bincount(dst, minlength=n_all).astype(np.int64)
    grp_bounds = None
    for limit in (1280, 1344, 1408, 1536):
        bounds = [0]
        acc = 0
        for n in range(N_NODES):        # fake nodes need no group
            c = int(ndeg[n])
            if (n - bounds[-1]) >= P or (acc + c > limit and n > bounds[-1]):
                bounds.append(n)
                acc = 0
            acc += c
        if len(bounds) <= NCORE * NT:
            grp_bounds = bounds
            break
    if grp_bounds is None:  # fallback: fixed 128-node groups
        grp_bounds = list(range(0, n_all, P))
    while len(grp_bounds) < NCORE * NT:
        grp_bounds.append(N_NODES)
    grp_bounds.append(N_NODES)
    grp_bounds = np.asarray(grp_bounds[: NCORE * NT + 1], dtype=np.int64)
    node_edge_start = np.concatenate([[0], np.cumsum(ndeg)])
    counts = (node_edge_start[grp_bounds[1:]]
              - node_edge_start[grp_bounds[:-1]])                   # [160]
    tile_start = node_edge_start[grp_bounds[:-1]]
    tile_start = np.concatenate([tile_start, [node_edge_start[n_all]]])
    order_g = np.argsort(-counts, kind="stable")                    # desc by edges
    # round r gets groups ranked [8r, 8r+8); core c takes the c-th of them
    assign = order_g.reshape(NT, NCORE)                             # [t, c] -> group
    counts_tc = counts[assign]                                      # [NT, NCORE]
    CH = np.maximum(1, np.ceil(counts_tc / P)).max(axis=1).astype(np.int64)  # [NT]
    c_total = int(CH.sum())
    pad8 = (-c_total) % 8
    CH[int(np.argmax(CH))] += 0  # keep shape
    CH[NT - 1] += pad8
    c_total += pad8
    e_pad = P * c_total
    chunk_base = np.concatenate([[0], np.cumsum(CH)])               # [NT+1]

    # per-core padded edge arrays; attr16 stored pre-transposed [16, e_pad]
    attr16 = np.zeros((NCORE, 16, e_pad), dtype=np.float16)
    lidx = np.full((NCORE, P, c_total), 300.0, dtype=np.float32)
    for c in range(NCORE):
        for t in range(NT):
            g = int(assign[t, c])
            s = int(node_edge_start[grp_bounds[g]])
            e = s + int(counts[g])
            m = e - s
            if m == 0:
                continue
            slot0 = P * int(chunk_base[t])
            attr16[c, 0:8, slot0 : slot0 + m] = ea_s[s:e].T.astype(np.float16)
            li = (dst_s[s:e] - int(grp_bounds[g])).astype(np.float32)
            idx = np.arange(m)
            lidx[c, idx % P, int(chunk_base[t]) + idx // P] = li

    deg_pad = np.zeros(n_all, np.float32)
    deg_pad[:N_NODES] = np.bincount(dst, minlength=N_NODES).astype(np.float32)[:N_NODES]
    batch_pad = np.full(n_all, 999.0, np.float32)
    batch_pad[:N_NODES] = batch.astype(np.float32)
    degrows = np.zeros((NCORE, NT, P), np.float32)
    batchcols = np.full((NCORE, P, NT), 999.0, np.float32)
    for c in range(NCORE):
        for t in range(NT):
            g = int(assign[t, c])
            n0, n1 = int(grp_bounds[g]), int(grp_bounds[g + 1])
            w = n1 - n0
            degrows[c, t, 0:w] = deg_pad[n0:n1]
            batchcols[c, 0:w, t] = batch_pad[n0:n1]

    gcounts = np.bincount(batch, minlength=N_GRAPHS).astype(np.float32)
    inv_counts = 1.0 / np.maximum(gcounts, 1.0)
    icmat = np.ascontiguousarray(np.broadcast_to(inv_counts[None, :], (HID, P))).astype(np.float32)

    iota128 = np.ascontiguousarray(
        np.broadcast_to(np.arange(P, dtype=np.float16)[None, :], (P, P))
    )
    iota_rep = np.ascontiguousarray(
        np.broadcast_to(
            np.tile(np.arange(P, dtype=np.float16), 8)[None, :], (P, 8 * P))
    )

    has_b2 = bool(np.any(b2 != 0.0))
    has_b1 = bool(np.any(b1 != 0.0))
    # block-diagonal double-w1: one K=32 matmul computes h1 for two
    # 512-edge blocks into the two partition halves of one PSUM tile
    w1s = np.zeros((32, P), dtype=np.float16)
    w1s[0:8, 0:HID] = w1.astype(np.float16)
    w1s[16:24, HID:P] = w1.astype(np.float16)
    # b1 bias applied twice (both partition halves of the packed L1 psum)
    b1two = np.concatenate([b1, b1]).astype(np.float32)[:, None]    # [128, 1]
    # block-diagonal w2: one K=128 matmul computes h2 for two chunks
    w2blk = np.zeros((P, P), np.float32)
    w2blk[0:HID, 0:HID] = w2
    w2blk[HID:P, HID:P] = w2
    b2bcast = np.ascontiguousarray(
        np.broadcast_to(np.tile(b2, 8)[None, :], (P, 8 * HID))).astype(np.float32)
    # rows: 0-63 w3_eff, 64 <- r0 (ones row), 65 <- b3_eff (deg row)
    w3aug = np.concatenate([w3_eff, r0[None, :], b3_eff[None, :]], axis=0)  # [66, 64]
    l1aug = np.concatenate([l1_w, l1_b[None, :]], axis=0)           # [65, 128]
    l2bcol = np.ascontiguousarray(l2_b[:, None])                    # [10, 1]

    shared = {
        "iota128": iota128,
        "iota_rep": iota_rep,
        "w1s": w1s,
        "b1two": b1two,
        "w2blk": w2blk.astype(np.float16),
        "b2bcast": b2bcast,
        "w3aug": w3aug.astype(np.float16),
        "l1aug": l1aug.astype(np.float16),
        "l2w": l2_w.astype(np.float16),
        "l2bcol": l2bcol.astype(np.float32),
        "icmat": icmat,
    }
    in_maps = []
    for c in range(NCORE):
        m = dict(shared)
        # pair-stack: rows 0-15 = even 512-block, rows 16-31 = odd block
        a = attr16[c].reshape(16, e_pad // 1024, 2, 512)
        m["attr16"] = np.ascontiguousarray(
            a.transpose(2, 0, 1, 3).reshape(32, e_pad // 2))
        m["lidx"] = np.ascontiguousarray(lidx[c]).astype(np.float16)
        m["degrows"] = np.ascontiguousarray(degrows[c]).astype(np.float16)
        m["batchcols"] = np.ascontiguousarray(batchcols[c]).astype(np.float16)
        in_maps.append(m)
    return in_maps, (tuple(int(x) for x in CH), has_b2, has_b1)


def _build(ch_key):
    if ch_key in _BUILD_CACHE:
        return _BUILD_CACHE[ch_key]
    CH = list(ch_key[0])
    has_b2 = ch_key[1]
    has_b1 = ch_key[2]
    c_total = sum(CH)
    e_pad = P * c_total
    nb1 = e_pad // 512

    nc = bacc.Bacc("TRN2", target_bir_lowering=False, debug=False,
                   num_devices=NCORE)

    d_attr = nc.dram_tensor("attr16", [32, e_pad // 2], F16, kind="ExternalInput")
    d_lidx = nc.dram_tensor("lidx", [P, c_total], F16, kind="ExternalInput")
    d_deg = nc.dram_tensor("degrows", [NT, P], F16, kind="ExternalInput")
    d_bat = nc.dram_tensor("batchcols", [P, NT], F16, kind="ExternalInput")
    d_iota = nc.dram_tensor("iota128", [P, P], F16, kind="ExternalInput")
    d_iotar = nc.dram_tensor("iota_rep", [P, 8 * P], F16, kind="ExternalInput")
    d_w1 = nc.dram_tensor("w1s", [32, P], F16, kind="ExternalInput")
    d_b1 = nc.dram_tensor("b1two", [P, 1], F32, kind="ExternalInput")
    d_w2 = nc.dram_tensor("w2blk", [P, P], F16, kind="ExternalInput")
    d_b2b = nc.dram_tensor("b2bcast", [P, 8 * HID], F32, kind="ExternalInput")
    d_w3 = nc.dram_tensor("w3aug", [HID + 2, HID], F16, kind="ExternalInput")
    d_l1 = nc.dram_tensor("l1aug", [HID + 1, 2 * HID], F16, kind="ExternalInput")
    d_l2 = nc.dram_tensor("l2w", [2 * HID, OUT_DIM], F16, kind="ExternalInput")
    d_l2b = nc.dram_tensor("l2bcol", [OUT_DIM, 1], F32, kind="ExternalInput")
    d_ic = nc.dram_tensor("icmat", [HID, P], F32, kind="ExternalInput")
    d_ccina = nc.dram_tensor("ccina", [HID, P], F32)
    d_ccouta = nc.dram_tensor("ccouta", [HID, P], F32, addr_space="Shared")
    d_ccinb = nc.dram_tensor("ccinb", [HID, P], F32)
    d_ccoutb = nc.dram_tensor("ccoutb", [HID, P], F32, addr_space="Shared")
    d_out = nc.dram_tensor("logitsT", [OUT_DIM, P], F32, kind="ExternalOutput")

    with tile.TileContext(nc) as tc, ExitStack() as ctx:
        const = ctx.enter_context(tc.tile_pool(name="const", bufs=1))

        attrT = const.tile([32, e_pad // 2], F16)
        # packed: rows 0-63 = h1 of even 512-block, rows 64-127 = odd block
        h1T2 = const.tile([P, e_pad // 2], F16)
        lidx_sb = const.tile([P, c_total], F16)
        iota_sb = const.tile([P, P], F16)
        iotar_sb = const.tile([P, 8 * P], F16)
        bat_sb = const.tile([P, NT], F16)
        w1_sb = const.tile([32, P], F16)
        b1_sb = const.tile([P, 1], F32)
        w2_sb = const.tile([P, P], F16)
        b2b_sb = const.tile([P, 8 * HID], F32)
        w3_sb = const.tile([HID + 2, HID], F16)
        l1_sb = const.tile([HID + 1, 2 * HID], F16)
        l2_sb = const.tile([2 * HID, OUT_DIM], F16)
        l2b_sb = const.tile([OUT_DIM, 1], F32)
        ic_sb = const.tile([HID, P], F32)
        # two persistent agga tiles (double-buffered by hand) so the ones
        # row is written once, not per node tile
        agga2 = [const.tile([HID + 2, P], F16, name=f"agga{i}", tag=f"agga{i}") for i in range(2)]

        n_sp = 4
        sp = e_pad // 2 // n_sp
        for s in range(n_sp):
            nc.sync.dma_start(attrT[:, s * sp : (s + 1) * sp],
                              d_attr[:, s * sp : (s + 1) * sp])
        nc.sync.dma_start(lidx_sb[:], d_lidx[:])
        nc.sync.dma_start(iota_sb[:], d_iota[:])
        nc.sync.dma_start(iotar_sb[:], d_iotar[:])
        nc.sync.dma_start(bat_sb[:], d_bat[:])
        nc.sync.dma_start(w1_sb[:], d_w1[:])
        nc.sync.dma_start(b1_sb[:], d_b1[:])
        nc.sync.dma_start(w2_sb[:], d_w2[:])
        if has_b2:
            nc.sync.dma_start(b2b_sb[:], d_b2b[:])
        nc.sync.dma_start(w3_sb[:], d_w3[:])
        nc.sync.dma_start(l1_sb[:], d_l1[:])
        nc.sync.dma_start(l2_sb[:], d_l2[:])
        nc.sync.dma_start(l2b_sb[:], d_l2b[:])
        nc.sync.dma_start(ic_sb[:], d_ic[:])
        nc.vector.memset(agga2[0][HID : HID + 1, :], 1.0)
        nc.vector.memset(agga2[1][HID : HID + 1, :], 1.0)
        # warmup operand needs no DMA
        warm_sb = const.tile([P, P], F16)
        nc.vector.memset(warm_sb[:], 0.125)


        with (
            tc.tile_pool(name="ps_pool", bufs=1, space="PSUM") as ps_pool,
            tc.tile_pool(name="ps_h1", bufs=2, space="PSUM") as ps_h1,
            tc.tile_pool(name="ps_h2", bufs=2, space="PSUM") as ps_h2,
            tc.tile_pool(name="ps_agg", bufs=2, space="PSUM") as ps_agg,
            tc.tile_pool(name="ps_out3", bufs=1, space="PSUM") as ps_out3,
            tc.tile_pool(name="work", bufs=3) as work,
            tc.tile_pool(name="ohpool", bufs=4) as ohpool,
        ):
            # all 20 pooling one-hots in one wide compare
            ohg_all = const.tile([P, NT * P], F16)
            nc.vector.tensor_tensor(
                ohg_all[:].rearrange("p (a j) -> p a j", a=NT),
                iota_sb[:].rearrange("p (a j) -> p a j", a=1).broadcast_to((P, NT, P)),
                bat_sb[:, 0:NT].broadcast_to((P, NT, P)),
                op=ALU.is_equal)

            # ---- PE warm-up: keep HAM busy while input DMAs land ----
            for wi in range(16):
                wps = ps_h1.tile([P, 512], F32, name="wps", tag="h1ps")
                nc.tensor.matmul(wps[:, 0:P], warm_sb[:], warm_sb[:],
                                 start=True, stop=True)

            # ---- L1 packed: block-diag w1 computes two 512-blocks/matmul ----
            for b in range(nb1 // 2):
                h1ps = ps_h1.tile([P, 512], F32, name="h1ps", tag="h1ps")
                nc.tensor.matmul(h1ps[:], w1_sb[:],
                                 attrT[:, b * 512 : (b + 1) * 512],
                                 start=True, stop=True)
                if has_b1 or b % 2 == 0:
                    nc.scalar.activation(h1T2[:, b * 512 : (b + 1) * 512],
                                         h1ps[:], AF.Relu, bias=b1_sb[:, 0:1])
                else:
                    nc.vector.tensor_scalar_max(
                        h1T2[:, b * 512 : (b + 1) * 512], h1ps[:], 0.0)

            # ---- L2 + scatter, per node tile ----
            n_groups = c_total // 8
            h2grp = [None] * n_groups
            # L2 in groups of 8 chunks sharing one PSUM bank & one relu op
            # one matmul per chunk-PAIR (chunks 8p+j and 8p+4+j share a
            # column window of h1T2; block-diag w2 separates them)
            for grp in range(n_groups):
                h2ps = ps_h2.tile([P, 512], F32)
                p_base = grp  # group == h1T2 512-col window == chunk pair block
                for j in range(4):
                    col = grp * 512 + j * P
                    nc.tensor.matmul(
                        h2ps[:, j * P : (j + 1) * P],
                        h1T2[:, col : col + P],
                        w2_sb[:],
                        start=True, stop=True)
                h2sb = work.tile([P, 512], F16, tag="h2sb")
                if has_b2:
                    h2tmp = work.tile([P, 512], F32, tag="h2tmp")
                    nc.vector.tensor_tensor(h2tmp[:], h2ps[:], b2b_sb[:],
                                            op=ALU.add)
                    nc.vector.tensor_scalar_max(h2sb[:], h2tmp[:], 0.0)
                else:
                    nc.scalar.activation(h2sb[:], h2ps[:], AF.Relu)
                h2grp[grp] = h2sb

            poolps = ps_pool.tile([HID, P], F32)
            n_ohb = c_total // 8
            ohbatch = [None] * n_ohb
            for ob in range(n_ohb):
                ohb = ohpool.tile([P, 8 * P], F16, tag="oh")
                nc.vector.tensor_tensor(
                    ohb[:].rearrange("p (a j) -> p a j", a=8),
                    iotar_sb[:].rearrange("p (a j) -> p a j", a=8),
                    lidx_sb[:, 8 * ob : 8 * ob + 8].broadcast_to((P, 8, P)),
                    op=ALU.is_equal)
                ohbatch[ob] = ohb

            kglob = 0
            for t in range(NT):
                aggps = ps_agg.tile([HID, P], F32)
                for j in range(CH[t]):
                    k = kglob + j
                    oh = ohbatch[k // 8][:, (k % 8) * P : (k % 8 + 1) * P]
                    h2sb = h2grp[k // 8]
                    hcol = P * (k % 4) + HID * ((k // 4) % 2)
                    nc.tensor.matmul(
                        aggps[:], h2sb[:, hcol : hcol + HID], oh[:],
                        start=(j == 0), stop=(j == CH[t] - 1))
                kglob += CH[t]

                # epilogue for node tile t
                agga = agga2[t % 2]
                nc.scalar.copy(agga[0:HID, :], aggps[:])
                nc.sync.dma_start(agga[HID + 1 : HID + 2, :],
                                  d_deg[t : t + 1, :])
                out3ps = ps_out3.tile([P, HID], F32, name="out3ps", tag="o3")
                nc.tensor.matmul(out3ps[:], agga[:], w3_sb[:], start=True, stop=True)
                outsb = work.tile([P, HID], F16, tag="outsb")
                nc.scalar.activation(outsb[:], out3ps[:], AF.Relu)
                nc.tensor.matmul(poolps[:], outsb[:],
                                 ohg_all[:, t * P : (t + 1) * P],
                                 start=(t == 0), stop=(t == NT - 1))

        pooled_sb = const.tile([HID, P], F32)
        nc.vector.tensor_copy(pooled_sb[:], poolps[:])
        nc.sync.dma_start(d_ccina[:], pooled_sb[:])
        nc.gpsimd.collective_compute(
            "AllReduce", ALU.add,
            replica_groups=[list(range(NCORE))],
            ins=[d_ccina[:]], outs=[d_ccouta[:]])

        with tc.tile_pool(name="ps_head", bufs=2, space="PSUM") as ps_head:
            pc_raw = const.tile([HID, P], F32)
            nc.sync.dma_start(pc_raw[:], d_ccouta[:])
            pc_aug = const.tile([HID + 1, P], F16)
            nc.vector.tensor_tensor(pc_aug[0:HID, :], pc_raw[:], ic_sb[:],
                                    op=ALU.mult)
            nc.vector.memset(pc_aug[HID : HID + 1, :], 1.0)
            zps = ps_head.tile([2 * HID, P], F32)
            nc.tensor.matmul(zps[:], l1_sb[:], pc_aug[:], start=True, stop=True)
            zsb = const.tile([2 * HID, P], F16)
            nc.scalar.activation(zsb[:], zps[:], AF.Relu)
            lps = ps_head.tile([OUT_DIM, P], F32)
            nc.tensor.matmul(lps[:], l2_sb[:], zsb[:], start=True, stop=True)
            lsb = const.tile([OUT_DIM, P], F32)
            nc.vector.tensor_scalar(lsb[:], lps[:], l2b_sb[:, 0:1], None,
                                    op0=ALU.add)
            nc.sync.dma_start(d_out[:], lsb[:])

    nc.compile()
    _BUILD_CACHE[ch_key] = nc
    return nc


def _run(inputs, trace=False, trace_cores=None, **kwargs):
    in_maps, ch_key = _preprocess(inputs)
    nc = _build(ch_key)
    res = run_bass_kernel_spmd(
        nc, in_maps, core_ids=list(range(NCORE)),
        trace=trace, trace_cores=trace_cores, **kwargs)
    return res


def kernel(**inputs):
    res = _run(inputs)
    logitsT = res.results[0]["logitsT"]
    return np.ascontiguousarray(logitsT.T).astype(np.float32)
